# revision 1
# baseline (speedup 1.0000x reference)
"""GQA causal attention (B=2, S=2048, D=2048, 16 q heads / 4 kv heads, RoPE)
for 8 Trainium2 NeuronCores.

Sharding: core i = (batch b = i//4, kv-head group g = i%4). Each core computes
its group's Q/K/V projections, RoPE, causal attention and the partial output
projection; the host sums the 4 per-group partials per batch.

On-core layout is fully "transposed" (features on partitions):
  xT [D, S], QT/KT [d, S] -> QK scores land as [k, q], softmax runs along k
  (partitions) with the denominator computed by an all-ones matmul, and PV
  accumulates out^T [d, q] directly in PSUM. The final projection contracts
  over the group's 512 head-dims on partitions.
All matmuls use float32r (full PE throughput at moving dim >= 256).
"""

import sys
import types

sys.path.insert(0, "/opt/trn_rl_repo")

# If tracing is ever requested (e.g. BASS_TRACE=1 in the environment),
# concourse needs antenv.axon_hooks, which this image lacks; provide it.
try:
    import antenv  # noqa: F401

    if "antenv.axon_hooks" not in sys.modules:
        from trn_agent_boot.trn_boot import _ntff_profile_via_ctypes

        _mod = types.ModuleType("antenv.axon_hooks")
        _hook = _ntff_profile_via_ctypes("/opt/axon/libaxon_pjrt.so")
        _mod.get_axon_ntff_profile_hook = lambda: _hook
        sys.modules["antenv.axon_hooks"] = _mod
except Exception:
    pass

import numpy as np
from contextlib import ExitStack

import concourse.bacc as bacc
import concourse.mybir as mybir
import concourse.tile as tile
from concourse.bass_utils import run_bass_kernel_spmd

B, S, DIM = 2, 2048, 2048
N_HEADS, N_KV, HD = 16, 4, 128
HPG = N_HEADS // N_KV      # q heads per kv group
GD = HPG * HD              # 512 = group width
P = 128
NS = S // 512              # 4 s-slices of 512
NC = DIM // P              # 16 contraction chunks of 128
NKT = S // P               # 16 k tiles
F32 = mybir.dt.float32
F32R = mybir.dt.float32r
SCALE = 1.0 / float(np.sqrt(HD))
MASK_NEG = -1.0e5

# consts column layout
C_RT = 0          # [128]  RoPE rotation (R.T)
C_ID = 128        # [128]  identity
C_ONES = 256      # [128]  all-ones
C_MASK = 384      # [4*512] causal band masks, additive
C_COS = 2432      # [2048] cos, repeated x2 along d
C_SIN = 4480      # [2048]
C_BM = 6528       # [4*512] causal band masks, multiplicative 0/1
NCONST = 8576

_CACHE = {}


def _build():
    nc = bacc.Bacc()
    xT = nc.dram_tensor("xT", [DIM, S], F32, kind="ExternalInput")
    wqT = nc.dram_tensor("wqT", [DIM, GD], F32, kind="ExternalInput")
    wkT = nc.dram_tensor("wkT", [DIM, HD], F32, kind="ExternalInput")
    wvT = nc.dram_tensor("wvT", [DIM, HD], F32, kind="ExternalInput")
    woT = nc.dram_tensor("woT", [GD, DIM], F32, kind="ExternalInput")
    consts = nc.dram_tensor("consts", [P, NCONST], F32, kind="ExternalInput")
    out = nc.dram_tensor("out", [S, DIM], F32, kind="ExternalOutput")

    EXP = mybir.ActivationFunctionType.Exp

    with tile.TileContext(nc) as tc, ExitStack() as ctx:
        cpool = ctx.enter_context(tc.tile_pool(name="consts", bufs=1))
        persist = ctx.enter_context(tc.tile_pool(name="persist", bufs=1))

        consts_sb = cpool.tile([P, NCONST], F32R, name="consts_sb")
        rt = consts_sb[:, C_RT:C_RT + 128]
        ident = consts_sb[:, C_ID:C_ID + 128].bitcast(F32)
        ones_r = consts_sb[:, C_ONES:C_ONES + 128]
        cosf = consts_sb[:, C_COS:C_COS + S].bitcast(F32)
        sinf = consts_sb[:, C_SIN:C_SIN + S].bitcast(F32)
        bmasks = consts_sb[:, C_BM:C_BM + 4 * 512].rearrange("p (r q) -> p r q", r=4)

        wo_sb = persist.tile([P, HPG, DIM], F32R, name="wo_sb")
        q_sb = persist.tile([P, HPG, S], F32R, name="q_sb")
        k_sb = persist.tile([P, S], F32R, name="k_sb")
        v_sb = persist.tile([P, NKT, HD], F32R, name="v_sb")

        # ---- Phase 1: QKV projections + RoPE + V transpose, per s-slice ----
        with ExitStack() as p1:
            wpool = p1.enter_context(tc.tile_pool(name="wqkv", bufs=1))
            xpool = p1.enter_context(tc.tile_pool(name="xs", bufs=3))
            vtpool = p1.enter_context(tc.tile_pool(name="vt", bufs=1))
            tmpp = p1.enter_context(tc.tile_pool(name="ropetmp", bufs=3))
            psA = p1.enter_context(tc.tile_pool(name="psA", bufs=1, space="PSUM"))
            psRT = p1.enter_context(tc.tile_pool(name="psRT", bufs=1, space="PSUM"))

            wq_sb = wpool.tile([P, NC, GD], F32R, name="wq_sb")
            wk_sb = wpool.tile([P, NC, HD], F32R, name="wk_sb")
            wv_sb = wpool.tile([P, NC, HD], F32R, name="wv_sb")
            vt_sb = vtpool.tile([P, S], F32, name="vt_sb")

            warm_sb = wpool.tile([P, 512], F32, name="warm_sb")
            nc.vector.memset(warm_sb, 0.0)
            for _ in range(8):
                wps = psRT.tile([P, 512], F32, name="warm_ps", tag="rt")
                nc.tensor.matmul(wps, warm_sb[:, :P], warm_sb, start=True, stop=True)

            def dma_consts(lo, n):
                nc.sync.dma_start(out=consts_sb[:, lo:lo + n],
                                  in_=consts[:, lo:lo + n].bitcast(F32R))

            def dma_wq_chunk(cc):
                nc.sync.dma_start(
                    out=wq_sb[:, 4 * cc:4 * (cc + 1), :],
                    in_=wqT[512 * cc:512 * (cc + 1), :]
                    .rearrange("(c p) h -> p c h", p=P).bitcast(F32R))

            def dma_xs(xs, j, cc):
                nc.sync.dma_start(
                    out=xs,
                    in_=xT[512 * cc:512 * (cc + 1), 512 * j:512 * (j + 1)]
                    .rearrange("(c p) s -> p c s", p=P).bitcast(F32R))

            prefetched = None
            for j in range(NS):
                sl = slice(512 * j, 512 * (j + 1))
                ps = [psA.tile([P, 512], F32, name=f"proj{t}",
                               bufs=2 if t == 0 else 1) for t in range(6)]
                j0_tiles = {}
                for cc in range(4):
                    if cc == 0 and j > 0:
                        xs = prefetched
                    elif j == 0 and cc in j0_tiles:
                        xs = j0_tiles[cc]
                    else:
                        xs = xpool.tile([P, 4, 512], F32R, name="xs")
                    if j == 0 and cc == 0:
                        # interleave the first x columns and q-weight chunks in
                        # small pieces, issued from three engines in parallel
                        # (SP descriptor generation is ~1us per DMA)
                        for c4 in range(4):
                            nc.sync.dma_start(
                                out=xs[:, c4, :],
                                in_=xT[128 * c4:128 * (c4 + 1), 0:512]
                                .rearrange("(c p) s -> p c s", p=P)[:, 0, :]
                                .bitcast(F32R))
                            nc.gpsimd.dma_start(
                                out=wq_sb[:, c4, :],
                                in_=wqT[128 * c4:128 * (c4 + 1), :]
                                .rearrange("(c p) h -> p c h", p=P)[:, 0, :]
                                .bitcast(F32R))
                        nc.scalar.dma_start(
                            out=wk_sb, in_=wkT[:, :]
                            .rearrange("(c p) h -> p c h", p=P).bitcast(F32R))
                        nc.scalar.dma_start(
                            out=wv_sb, in_=wvT[:, :]
                            .rearrange("(c p) h -> p c h", p=P).bitcast(F32R))
                        nc.gpsimd.dma_start(
                            out=consts_sb[:, C_RT:C_RT + 256],
                            in_=consts[:, C_RT:C_RT + 256].bitcast(F32R))
                        nc.scalar.dma_start(
                            out=consts_sb[:, C_COS:C_COS + 512],
                            in_=consts[:, C_COS:C_COS + 512].bitcast(F32R))
                        nc.gpsimd.dma_start(
                            out=consts_sb[:, C_SIN:C_SIN + 512],
                            in_=consts[:, C_SIN:C_SIN + 512].bitcast(F32R))
                    elif not (j == 0 and cc in j0_tiles) and not (cc == 0 and j > 0):
                        dma_xs(xs, j, cc)
                    if j == 0 and cc + 1 < 4 and cc + 1 not in j0_tiles:
                        nxt = xpool.tile([P, 4, 512], F32R, name="xs")
                        dma_xs(nxt, 0, cc + 1)
                        dma_wq_chunk(cc + 1)
                        j0_tiles[cc + 1] = nxt
                    for c4 in range(4):
                        c = 4 * cc + c4
                        first = c == 0
                        last = c == NC - 1
                        for t in range(HPG):
                            nc.tensor.matmul(
                                ps[t], wq_sb[:, c, 128 * t:128 * (t + 1)],
                                xs[:, c4, :], start=first, stop=last)
                        nc.tensor.matmul(ps[4], wk_sb[:, c, :], xs[:, c4, :],
                                         start=first, stop=last)
                        nc.tensor.matmul(ps[5], wv_sb[:, c, :], xs[:, c4, :],
                                         start=first, stop=last)
                if j + 1 < NS:
                    xs_pre = xpool.tile([P, 4, 512], F32R, name="xs")
                    dma_xs(xs_pre, j + 1, 0)
                    prefetched = xs_pre
                else:
                    prefetched = None
                for t in range(HPG):
                    nc.scalar.copy(q_sb[:, t, sl], ps[t])
                nc.scalar.copy(k_sb[:, sl], ps[4])
                nc.scalar.copy(vt_sb[:, sl], ps[5])
                if j == 0:
                    # attention constants + next slice's cos/sin
                    dma_consts(C_ONES, 128)
                    dma_consts(C_BM, 4 * 512)
                elif j == 1:
                    nc.sync.dma_start(
                        out=wo_sb,
                        in_=woT[:, :].rearrange("(c p) e -> p c e", p=P).bitcast(F32R))
                if j + 1 < NS:
                    dma_consts(C_COS + 512 * (j + 1), 512)
                    dma_consts(C_SIN + 512 * (j + 1), 512)

                # RoPE for this slice (4 q heads + k)
                for t in range(HPG + 1):
                    src = q_sb[:, t, sl] if t < HPG else k_sb[:, sl]
                    t2 = tmpp.tile([P, 512], F32, name="t2")
                    nc.vector.tensor_mul(t2, src.bitcast(F32), cosf[:, sl])
                    rot = psRT.tile([P, 512], F32, name="rot", tag="rt")
                    nc.tensor.matmul(rot, rt, src, start=True, stop=True)
                    t1 = tmpp.tile([P, 512], F32, name="t1")
                    nc.vector.tensor_mul(t1, rot, sinf[:, sl])
                    nc.vector.tensor_add(src, t1, t2)

                # V transpose for this slice's 4 k-tiles
                for kt in range(4 * j, 4 * (j + 1)):
                    tr = psRT.tile([P, 512], F32, name="tr", tag="rt")[:, :P]
                    nc.tensor.transpose(tr, vt_sb[:, P * kt:P * (kt + 1)], ident)
                    nc.scalar.copy(v_sb[:, kt, :], tr)

        # ---- Phase 2: attention (j outer) + interleaved output projection ----
        with ExitStack() as p3:
            ppool = p3.enter_context(tc.tile_pool(name="ptiles", bufs=10))
            bcpool = p3.enter_context(tc.tile_pool(name="bc", bufs=3))
            attnp = p3.enter_context(tc.tile_pool(name="attn", bufs=1))
            outp = p3.enter_context(tc.tile_pool(name="outp", bufs=6))
            psQK = p3.enter_context(tc.tile_pool(name="psQK", bufs=3, space="PSUM"))
            psPV = p3.enter_context(tc.tile_pool(name="psPV", bufs=2, space="PSUM"))
            psDN = p3.enter_context(tc.tile_pool(name="psDN", bufs=1, space="PSUM"))
            psO = p3.enter_context(tc.tile_pool(name="psO", bufs=2, space="PSUM"))

            attn_sb = attnp.tile([P, HPG, S], F32R, name="attn_sb")

            for j in range(NS):
                sl = slice(512 * j, 512 * (j + 1))
                nkt = 4 * (j + 1)
                for h in range(HPG):
                    pv = psPV.tile([P, 512], F32, name="pv")
                    den = psDN.tile([P, 512], F32, name="den")
                    pts = [None] * nkt

                    # band tiles r=1,2 have their first 128r q-columns fully
                    # masked; skip those columns (fp32r needs moving >= 256, so
                    # r=3 stays full width)
                    def qlo(kt):
                        r = kt - 4 * j
                        return 128 * r if r in (1, 2) else 0

                    # Band (masked diagonal) tiles first: their longer pt
                    # chains (exp + gpsimd mask) hide behind later matmuls.
                    # Full tiles' denominators are pre-summed on the DVE so the
                    # PE runs one denominator matmul per pair instead of per
                    # tile.
                    order = list(range(4 * j, nkt)) + list(range(0, 4 * j))
                    nfull = 4 * j
                    dsums = [None]

                    def score(kt):
                        lo = qlo(kt)
                        qk = psQK.tile([P, 512], F32, name="qk")
                        nc.tensor.matmul(qk[:, lo:], k_sb[:, P * kt:P * (kt + 1)],
                                         q_sb[:, h, 512 * j + lo:512 * (j + 1)],
                                         start=True, stop=True)
                        pt = ppool.tile([P, 512], F32R, name="pt")
                        nc.scalar.activation(pt[:, lo:], qk[:, lo:], EXP, scale=SCALE)
                        r = kt - 4 * j
                        if r >= 0:
                            nc.gpsimd.tensor_mul(pt[:, lo:], pt[:, lo:],
                                                 bmasks[:, r, lo:])
                        pts[kt] = pt

                    def accum(i):
                        kt = order[i]
                        lo = qlo(kt)
                        nc.tensor.matmul(pv[:, lo:], v_sb[:, kt, :], pts[kt][:, lo:],
                                         start=(i == 0), stop=(i == nkt - 1))
                        if i < 4:   # band tile: individual denominator matmul
                            nc.tensor.matmul(den[:, lo:], ones_r, pts[kt][:, lo:],
                                             start=(i == 0), stop=(i == nkt - 1))
                        elif (i - 4) % 2 == 1:  # second of a full-tile pair
                            s = bcpool.tile([P, 512], F32R, name="densum")
                            nc.vector.tensor_add(s, pts[order[i - 1]], pts[kt])
                            nc.tensor.matmul(den, ones_r, s,
                                             start=False, stop=(i == nkt - 1))

                    score(order[0])
                    for i in range(1, nkt):
                        score(order[i])
                        accum(i - 1)
                    accum(nkt - 1)

                    rec_sb = bcpool.tile([P, 512], F32, name="rec_sb")
                    nc.vector.reciprocal_approx_fast(rec_sb, den)
                    nc.vector.tensor_mul(attn_sb[:, h, sl], pv, rec_sb)

                # output projection for the 4 s-tiles completed by this slice
                for st in range(4 * j, 4 * (j + 1)):
                    for e in range(NS):
                        ops = psO.tile([P, 512], F32, name="ops")
                        for hc in range(HPG):
                            nc.tensor.matmul(
                                ops, attn_sb[:, hc, P * st:P * (st + 1)],
                                wo_sb[:, hc, 512 * e:512 * (e + 1)],
                                start=(hc == 0), stop=(hc == HPG - 1))
                        osb = outp.tile([P, 512], F32, name="osb")
                        nc.vector.tensor_copy(osb, ops)
                        nc.sync.dma_start(
                            out=out[P * st:P * (st + 1), 512 * e:512 * (e + 1)],
                            in_=osb)

    nc.compile()
    return nc


def _consts_array(freqs_cos, freqs_sin):
    c = np.zeros((P, NCONST), np.float32)
    rt = np.zeros((P, P), np.float32)
    idx = np.arange(0, P, 2)
    rt[idx, idx + 1] = 1.0    # (R.T)[2j, 2j+1] = +1
    rt[idx + 1, idx] = -1.0   # (R.T)[2j+1, 2j] = -1
    c[:, C_RT:C_RT + P] = rt
    c[:, C_ID:C_ID + P] = np.eye(P, dtype=np.float32)
    c[:, C_ONES:C_ONES + P] = 1.0
    ki = np.arange(P)[:, None]
    qi = np.arange(512)[None, :]
    for r in range(4):
        c[:, C_MASK + 512 * r:C_MASK + 512 * (r + 1)] = np.where(
            ki <= qi - P * r, 0.0, MASK_NEG).astype(np.float32)
    c[:, C_COS:C_COS + S] = np.repeat(np.asarray(freqs_cos, np.float32).T, 2, axis=0)
    c[:, C_SIN:C_SIN + S] = np.repeat(np.asarray(freqs_sin, np.float32).T, 2, axis=0)
    for r in range(4):
        c[:, C_BM + 512 * r:C_BM + 512 * (r + 1)] = (ki <= qi - P * r).astype(np.float32)
    return c


def _in_maps(x, wq, wk, wv, wo, freqs_cos, freqs_sin):
    x = np.asarray(x, np.float32)
    wq = np.asarray(wq, np.float32)
    wk = np.asarray(wk, np.float32)
    wv = np.asarray(wv, np.float32)
    wo = np.asarray(wo, np.float32)
    consts = _consts_array(freqs_cos, freqs_sin)
    maps = []
    for core in range(8):
        b, g = divmod(core, 4)
        maps.append({
            "xT": np.ascontiguousarray(x[b].T),
            "wqT": np.ascontiguousarray(wq[GD * g:GD * (g + 1), :].T),
            "wkT": np.ascontiguousarray(wk[HD * g:HD * (g + 1), :].T),
            "wvT": np.ascontiguousarray(wv[HD * g:HD * (g + 1), :].T),
            "woT": np.ascontiguousarray(wo[:, GD * g:GD * (g + 1)].T),
            "consts": consts,
        })
    return maps


def _get_nc():
    if "nc" not in _CACHE:
        _CACHE["nc"] = _build()
    return _CACHE["nc"]


def _run(in_maps, trace=False):
    return run_bass_kernel_spmd(_get_nc(), in_maps, core_ids=list(range(8)),
                                trace=trace)


def kernel(x, wq, wk, wv, wo, freqs_cos, freqs_sin):
    res = _run(_in_maps(x, wq, wk, wv, wo, freqs_cos, freqs_sin))
    out = np.zeros((B, S, DIM), np.float32)
    for core in range(8):
        b = core // 4
        out[b] += res.results[core]["out"]
    return out



# revision 3
# speedup vs baseline: 1.0571x; 1.0571x over previous
"""GQA causal attention (B=2, S=2048, D=2048, 16 q heads / 4 kv heads, RoPE)
for 8 Trainium2 NeuronCores.

Sharding: core i = (batch b = i//4, kv-head group g = i%4). Each core computes
its group's Q/K/V projections, RoPE, causal attention and the partial output
projection; the host sums the 4 per-group partials per batch.

On-core layout is fully "transposed" (features on partitions):
  xT [D, S], QT/KT [d, S] -> QK scores land as [k, q], softmax runs along k
  (partitions) with the denominator computed by an all-ones matmul, and PV
  accumulates out^T [d, q] directly in PSUM. The final projection contracts
  over the group's 512 head-dims on partitions.

Everything is bf16 except PSUM accumulation, the exp input (fp32 scores in
PSUM) and the RoPE temporaries. The whole kernel is one fused pipeline:
per 512-token slice j we emit QKV projection -> RoPE -> V transpose ->
attention (k-tiles 0..4(j+1)) -> output projection, so the tensor engine
never drains between "phases". QK scores for two k-tiles share one 2-bank
PSUM tile so exp runs 1024 wide; softmax denominators are tree-summed on the
DVE (4 tiles -> 1 ones-matmul).
"""

import sys
import types

sys.path.insert(0, "/opt/trn_rl_repo")

# If tracing is ever requested (e.g. BASS_TRACE=1 in the environment),
# concourse needs antenv.axon_hooks, which this image lacks; provide it.
try:
    import antenv  # noqa: F401

    if "antenv.axon_hooks" not in sys.modules:
        from trn_agent_boot.trn_boot import _ntff_profile_via_ctypes

        _mod = types.ModuleType("antenv.axon_hooks")
        _hook = _ntff_profile_via_ctypes("/opt/axon/libaxon_pjrt.so")
        _mod.get_axon_ntff_profile_hook = lambda: _hook
        sys.modules["antenv.axon_hooks"] = _mod
except Exception:
    pass

import numpy as np
import ml_dtypes
from contextlib import ExitStack

import concourse.bacc as bacc
import concourse.mybir as mybir
import concourse.tile as tile
from concourse.bass_utils import run_bass_kernel_spmd

B, S, DIM = 2, 2048, 2048
N_HEADS, N_KV, HD = 16, 4, 128
HPG = N_HEADS // N_KV      # q heads per kv group
GD = HPG * HD              # 512 = group width
P = 128
NS = S // 512              # 4 s-slices of 512
NC = DIM // P              # 16 contraction chunks of 128
NKT = S // P               # 16 k tiles
F32 = mybir.dt.float32
BF16 = mybir.dt.bfloat16
BF = ml_dtypes.bfloat16
SCALE = 1.0 / float(np.sqrt(HD))

# bf16 consts column layout
C_RT = 0          # [128]  RoPE rotation (R.T)
C_ID = 128        # [128]  identity
C_ONES = 256      # [128]  all-ones
C_COS = 384       # [2048] cos, repeated x2 along d
C_BM = 2432       # [4*512] causal band masks, multiplicative 0/1
NC16 = 4480

_CACHE = {}


def _build():
    nc = bacc.Bacc()
    xT = nc.dram_tensor("xT", [DIM, S], BF16, kind="ExternalInput")
    wqT = nc.dram_tensor("wqT", [DIM, GD], BF16, kind="ExternalInput")
    wkT = nc.dram_tensor("wkT", [DIM, HD], BF16, kind="ExternalInput")
    wvT = nc.dram_tensor("wvT", [DIM, HD], BF16, kind="ExternalInput")
    woT = nc.dram_tensor("woT", [GD, DIM], BF16, kind="ExternalInput")
    c16 = nc.dram_tensor("c16", [P, NC16], BF16, kind="ExternalInput")
    c32 = nc.dram_tensor("c32", [P, S], F32, kind="ExternalInput")
    out = nc.dram_tensor("out", [S, DIM], BF16, kind="ExternalOutput")

    EXP = mybir.ActivationFunctionType.Exp

    with tile.TileContext(nc) as tc, ExitStack() as ctx:
        cpool = ctx.enter_context(tc.tile_pool(name="consts", bufs=1))
        persist = ctx.enter_context(tc.tile_pool(name="persist", bufs=1))
        xpool = ctx.enter_context(tc.tile_pool(name="xs", bufs=8))
        vtpool = ctx.enter_context(tc.tile_pool(name="vt", bufs=2))
        tmpp = ctx.enter_context(tc.tile_pool(name="ropetmp", bufs=4))
        ptp = ctx.enter_context(tc.tile_pool(name="pt", bufs=5))
        dsp = ctx.enter_context(tc.tile_pool(name="ds", bufs=4))
        recp = ctx.enter_context(tc.tile_pool(name="rec", bufs=2))
        outp = ctx.enter_context(tc.tile_pool(name="outp", bufs=4))
        # PSUM: psP 2x[P,512] (projection chains + softmax denominator),
        # psQ 2x[P,1024] (QK score pairs; also rot/transpose/out-proj),
        # psV 2x[P,512] (PV accumulators) = 8 banks exactly.
        psP = ctx.enter_context(tc.tile_pool(name="psP", bufs=2, space="PSUM"))
        psQ = ctx.enter_context(tc.tile_pool(name="psQ", bufs=2, space="PSUM"))
        psV = ctx.enter_context(tc.tile_pool(name="psV", bufs=2, space="PSUM"))

        c16_sb = cpool.tile([P, NC16], BF16, name="c16_sb")
        sin_sb = cpool.tile([P, S], F32, name="sin_sb")
        rt = c16_sb[:, C_RT:C_RT + 128]
        ident = c16_sb[:, C_ID:C_ID + 128]
        ones = c16_sb[:, C_ONES:C_ONES + 128]
        cosf = c16_sb[:, C_COS:C_COS + S]
        bm = c16_sb[:, C_BM:C_BM + 2048]

        wq_sb = persist.tile([P, NC, GD], BF16, name="wq_sb")
        wk_sb = persist.tile([P, NC, HD], BF16, name="wk_sb")
        wv_sb = persist.tile([P, NC, HD], BF16, name="wv_sb")
        wo_sb = persist.tile([P, HPG, DIM], BF16, name="wo_sb")
        q_sb = persist.tile([P, HPG, S], BF16, name="q_sb")
        k_sb = persist.tile([P, S], BF16, name="k_sb")
        v_sb = persist.tile([P, NKT, HD], BF16, name="v_sb")
        attn_sb = persist.tile([P, HPG, S], BF16, name="attn_sb")
        warm_sb = persist.tile([P, 512], BF16, name="warm_sb")

        # ---- warmup: keep PE busy (HAM un-throttle) while first DMAs land
        nc.gpsimd.memset(warm_sb, 0.0)
        for _ in range(10):
            wps = psQ.tile([P, 512], F32, name="warm_ps", tag="qk")
            nc.tensor.matmul(wps, warm_sb[:, :P], warm_sb, start=True, stop=True)

        # ---- startup DMAs; smallest/most-urgent first on each queue ----
        # gpsimd queue: K and V weights (first projection targets), consts
        nc.gpsimd.dma_start(out=wk_sb,
                            in_=wkT.rearrange("(c p) h -> p c h", p=P))
        nc.gpsimd.dma_start(out=wv_sb,
                            in_=wvT.rearrange("(c p) h -> p c h", p=P))
        nc.gpsimd.dma_start(out=c16_sb[:, 0:C_COS + 512],
                            in_=c16[:, 0:C_COS + 512])

        # scalar queue: sin slice 0, then halves of wq
        nc.scalar.dma_start(out=sin_sb[:, 0:512], in_=c32[:, 0:512])

        def dma_wq(cc, eng):
            eng.dma_start(
                out=wq_sb[:, 4 * cc:4 * (cc + 1), :],
                in_=wqT[512 * cc:512 * (cc + 1), :]
                .rearrange("(c p) h -> p c h", p=P))

        dma_wq(0, nc.gpsimd)
        dma_wq(1, nc.gpsimd)
        dma_wq(2, nc.scalar)
        dma_wq(3, nc.scalar)

        # sync queue: x chunk 0 of slice 0 in 4 pieces (fast first tile),
        # then the remaining chunks
        xs_j = {0: []}
        xs0 = xpool.tile([P, 4, 512], BF16, name="xs", tag="xs")
        for c4 in range(4):
            nc.sync.dma_start(out=xs0[:, c4, :],
                              in_=xT[128 * c4:128 * (c4 + 1), 0:512])
        xs_j[0].append(xs0)
        for cc in range(1, 4):
            t = xpool.tile([P, 4, 512], BF16, name="xs", tag="xs")
            nc.sync.dma_start(
                out=t,
                in_=xT[512 * cc:512 * (cc + 1), 0:512]
                .rearrange("(c p) s -> p c s", p=P))
            xs_j[0].append(t)

        dma_engs = [nc.sync, nc.gpsimd, nc.scalar]
        out_dma_rr = [0]

        for j in range(NS):
            sl = slice(512 * j, 512 * (j + 1))
            xs = xs_j[j]

            # prefetch x chunks for slice j+1 (sync queue, behind slice j)
            if j + 1 < NS:
                nxt = []
                for cc in range(4):
                    t = xpool.tile([P, 4, 512], BF16, name="xs", tag="xs")
                    nc.sync.dma_start(
                        out=t,
                        in_=xT[512 * cc:512 * (cc + 1),
                               512 * (j + 1):512 * (j + 2)]
                        .rearrange("(c p) s -> p c s", p=P))
                    nxt.append(t)
                xs_j[j + 1] = nxt
                # cos/sin for slice j+1
                nc.scalar.dma_start(
                    out=c16_sb[:, C_COS + 512 * (j + 1):C_COS + 512 * (j + 2)],
                    in_=c16[:, C_COS + 512 * (j + 1):C_COS + 512 * (j + 2)])
                nc.scalar.dma_start(
                    out=sin_sb[:, 512 * (j + 1):512 * (j + 2)],
                    in_=c32[:, 512 * (j + 1):512 * (j + 2)])
            if j == 0:
                # band masks + output weights, needed from attention onwards
                nc.gpsimd.dma_start(out=c16_sb[:, C_BM:C_BM + 2048],
                                    in_=c16[:, C_BM:C_BM + 2048])
                nc.scalar.dma_start(
                    out=wo_sb, in_=woT.rearrange("(c p) e -> p c e", p=P))

            # ---- QKV projections: K first, then V, then the 4 Q heads ----
            ps_k = psP.tile([P, 512], F32, name="psp", tag="pp")
            for c in range(NC):
                nc.tensor.matmul(ps_k, wk_sb[:, c, :], xs[c // 4][:, c % 4, :],
                                 start=(c == 0), stop=(c == NC - 1))
            nc.scalar.copy(k_sb[:, sl], ps_k)

            ps_v = psP.tile([P, 512], F32, name="psp", tag="pp")
            for c in range(NC):
                nc.tensor.matmul(ps_v, wv_sb[:, c, :], xs[c // 4][:, c % 4, :],
                                 start=(c == 0), stop=(c == NC - 1))
            vt = vtpool.tile([P, 512], BF16, name="vt_sb")
            nc.scalar.copy(vt, ps_v)

            for t in range(HPG):
                ps_q = psP.tile([P, 512], F32, name="psp", tag="pp")
                for c in range(NC):
                    nc.tensor.matmul(ps_q, wq_sb[:, c, 128 * t:128 * (t + 1)],
                                     xs[c // 4][:, c % 4, :],
                                     start=(c == 0), stop=(c == NC - 1))
                nc.scalar.copy(q_sb[:, t, sl], ps_q)

            # ---- RoPE for k then q0..q3 ----
            for t in range(HPG + 1):
                src = k_sb[:, sl] if t == 0 else q_sb[:, t - 1, sl]
                t2 = tmpp.tile([P, 512], F32, name="t2")
                nc.vector.tensor_mul(t2, src, cosf[:, sl])
                rot = psQ.tile([P, 512], F32, name="rot", tag="qk")
                nc.tensor.matmul(rot, rt, src, start=True, stop=True)
                t1 = tmpp.tile([P, 512], F32, name="t1")
                nc.vector.tensor_mul(t1, rot, sin_sb[:, sl])
                nc.vector.tensor_add(src, t1, t2)

            # ---- V transpose for this slice's 4 k-tiles ----
            for i in range(4):
                tr = psQ.tile([P, 512], BF16, name="tr", tag="qk")
                nc.tensor.transpose(tr[:, :P], vt[:, P * i:P * (i + 1)], ident)
                nc.vector.tensor_copy(v_sb[:, 4 * j + i, :], tr[:, :P])

            # ---- attention for slice j ----
            nkt = 4 * (j + 1)
            npair = nkt // 2
            for h in range(HPG):
                q_hi = q_sb[:, h, sl]
                pv = psV.tile([P, 512], F32, name="pv")
                den = psP.tile([P, 512], F32, name="den", tag="pp")
                # pairs of k-tiles; diagonal (masked) pairs first so their
                # longer exp+mask chains hide behind later matmuls
                pairs = ([(4 * j, 4 * j + 1, 0), (4 * j + 2, 4 * j + 3, 1)]
                         + [(2 * m, 2 * m + 1, None) for m in range(2 * j)])
                pts = [None] * npair
                dsums = [None] * npair

                def score(p):
                    kt0, kt1, bi = pairs[p]
                    qk = psQ.tile([P, 1024], F32, name="qk", tag="qk")
                    nc.tensor.matmul(qk[:, 0:512], k_sb[:, P * kt0:P * (kt0 + 1)],
                                     q_hi, start=True, stop=True)
                    nc.tensor.matmul(qk[:, 512:1024], k_sb[:, P * kt1:P * (kt1 + 1)],
                                     q_hi, start=True, stop=True)
                    pt = ptp.tile([P, 1024], BF16, name="pt")
                    nc.scalar.activation(pt, qk, EXP, scale=SCALE)
                    if bi is not None:
                        nc.vector.tensor_mul(pt, pt,
                                             bm[:, 1024 * bi:1024 * (bi + 1)])
                    pts[p] = pt

                def accum(p):
                    kt0, kt1, bi = pairs[p]
                    for z, kt in enumerate((kt0, kt1)):
                        r = kt - 4 * j
                        lo = 128 * r if (bi is not None and r >= 1) else 0
                        nc.tensor.matmul(pv[:, lo:], v_sb[:, kt, :],
                                         pts[p][:, 512 * z + lo:512 * (z + 1)],
                                         start=(p == 0 and z == 0),
                                         stop=(p == npair - 1 and z == 1))
                    ds = dsp.tile([P, 512], BF16, name="ds")
                    nc.vector.tensor_add(ds, pts[p][:, 0:512], pts[p][:, 512:1024])
                    dsums[p] = ds
                    if p % 2 == 1:
                        sq = dsp.tile([P, 512], BF16, name="sq")
                        nc.vector.tensor_add(sq, dsums[p - 1], ds)
                        nc.tensor.matmul(den, ones, sq,
                                         start=(p == 1), stop=(p == npair - 1))

                score(0)
                for p in range(1, npair):
                    score(p)
                    accum(p - 1)
                accum(npair - 1)

                rec = recp.tile([P, 512], F32, name="rec")
                nc.vector.reciprocal_approx_fast(rec, den)
                nc.vector.tensor_mul(attn_sb[:, h, sl], pv, rec)

            # ---- output projection for the 4 s-tiles of this slice ----
            for st in range(4 * j, 4 * (j + 1)):
                for e in range(NS):
                    ops = psQ.tile([P, 512], F32, name="ops", tag="qk")
                    for hc in range(HPG):
                        nc.tensor.matmul(
                            ops, attn_sb[:, hc, P * st:P * (st + 1)],
                            wo_sb[:, hc, 512 * e:512 * (e + 1)],
                            start=(hc == 0), stop=(hc == HPG - 1))
                    osb = outp.tile([P, 512], BF16, name="osb")
                    nc.vector.tensor_copy(osb, ops)
                    eng = dma_engs[out_dma_rr[0] % 3]
                    out_dma_rr[0] += 1
                    eng.dma_start(
                        out=out[P * st:P * (st + 1), 512 * e:512 * (e + 1)],
                        in_=osb)

    nc.compile()
    return nc


def _consts16(freqs_cos):
    c = np.zeros((P, NC16), np.float32)
    rtm = np.zeros((P, P), np.float32)
    idx = np.arange(0, P, 2)
    rtm[idx, idx + 1] = 1.0    # (R.T)[2j, 2j+1] = +1
    rtm[idx + 1, idx] = -1.0   # (R.T)[2j+1, 2j] = -1
    c[:, C_RT:C_RT + P] = rtm
    c[:, C_ID:C_ID + P] = np.eye(P, dtype=np.float32)
    c[:, C_ONES:C_ONES + P] = 1.0
    c[:, C_COS:C_COS + S] = np.repeat(np.asarray(freqs_cos, np.float32).T, 2,
                                      axis=0)
    ki = np.arange(P)[:, None]
    qi = np.arange(512)[None, :]
    for r in range(4):
        c[:, C_BM + 512 * r:C_BM + 512 * (r + 1)] = \
            (ki <= qi - P * r).astype(np.float32)
    return c.astype(BF)


def _in_maps(x, wq, wk, wv, wo, freqs_cos, freqs_sin):
    x = np.asarray(x, np.float32)
    wq = np.asarray(wq, np.float32)
    wk = np.asarray(wk, np.float32)
    wv = np.asarray(wv, np.float32)
    wo = np.asarray(wo, np.float32)
    c16a = _consts16(freqs_cos)
    c32a = np.ascontiguousarray(
        np.repeat(np.asarray(freqs_sin, np.float32).T, 2, axis=0))
    maps = []
    for core in range(8):
        b, g = divmod(core, 4)
        maps.append({
            "xT": np.ascontiguousarray(x[b].T).astype(BF),
            "wqT": np.ascontiguousarray(wq[GD * g:GD * (g + 1), :].T).astype(BF),
            "wkT": np.ascontiguousarray(wk[HD * g:HD * (g + 1), :].T).astype(BF),
            "wvT": np.ascontiguousarray(wv[HD * g:HD * (g + 1), :].T).astype(BF),
            "woT": np.ascontiguousarray(wo[:, GD * g:GD * (g + 1)].T).astype(BF),
            "c16": c16a,
            "c32": c32a,
        })
    return maps


def _get_nc():
    if "nc" not in _CACHE:
        _CACHE["nc"] = _build()
    return _CACHE["nc"]


def _run(in_maps, trace=False):
    return run_bass_kernel_spmd(_get_nc(), in_maps, core_ids=list(range(8)),
                                trace=trace)


def kernel(x, wq, wk, wv, wo, freqs_cos, freqs_sin):
    res = _run(_in_maps(x, wq, wk, wv, wo, freqs_cos, freqs_sin))
    out = np.zeros((B, S, DIM), np.float32)
    for core in range(8):
        b = core // 4
        out[b] += res.results[core]["out"].astype(np.float32)
    return out


# revision 6
# speedup vs baseline: 1.1650x; 1.1021x over previous
"""GQA causal attention (B=2, S=2048, D=2048, 16 q heads / 4 kv heads, RoPE)
for 8 Trainium2 NeuronCores.

Sharding: core i = (batch b = i//4, kv-head group g = i%4). Each core computes
its group's Q/K/V projections, RoPE, causal attention and the partial output
projection; the host sums the 4 per-group partials per batch.

On-core layout is fully "transposed" (features on partitions):
  xT [D, S], QT/KT [d, S] -> QK scores land as [k, q], softmax runs along k
  (partitions) with the denominator computed by an all-ones matmul, and PV
  accumulates out^T [d, q] directly in PSUM. The final projection contracts
  over the group's 512 head-dims on partitions.

Everything is bf16 except PSUM accumulation, the exp input (fp32 scores in
PSUM) and the RoPE temporaries. The whole kernel is one fused pipeline:
per 512-token slice j we emit {K proj, RoPE(k), V proj, V transpose,
Q proj + RoPE(q) per head, attention over k-tiles 0..4(j+1), output
projection}, so the tensor engine never drains between "phases". QK scores
for two k-tiles share one 2-bank PSUM tile so exp runs 1024 wide; softmax
denominators are tree-summed on the DVE (4 tiles -> 1 ones-matmul). All
HBM operands are pre-swizzled on the host so every DMA moves 4-16KB of
contiguous bytes per partition row.
"""

import sys
import types

sys.path.insert(0, "/opt/trn_rl_repo")

# If tracing is ever requested (e.g. BASS_TRACE=1 in the environment),
# concourse needs antenv.axon_hooks, which this image lacks; provide it.
try:
    import antenv  # noqa: F401

    if "antenv.axon_hooks" not in sys.modules:
        from trn_agent_boot.trn_boot import _ntff_profile_via_ctypes

        _mod = types.ModuleType("antenv.axon_hooks")
        _hook = _ntff_profile_via_ctypes("/opt/axon/libaxon_pjrt.so")
        _mod.get_axon_ntff_profile_hook = lambda: _hook
        sys.modules["antenv.axon_hooks"] = _mod
except Exception:
    pass

import numpy as np
import ml_dtypes
from contextlib import ExitStack

import concourse.bacc as bacc
import concourse.mybir as mybir
import concourse.tile as tile
from concourse.bass_utils import run_bass_kernel_spmd

B, S, DIM = 2, 2048, 2048
N_HEADS, N_KV, HD = 16, 4, 128
HPG = N_HEADS // N_KV      # q heads per kv group
GD = HPG * HD              # 512 = group width
P = 128
NS = S // 512              # 4 s-slices of 512
NC = DIM // P              # 16 contraction chunks of 128
NKT = S // P               # 16 k tiles
F32 = mybir.dt.float32
BF16 = mybir.dt.bfloat16
BF = ml_dtypes.bfloat16
SCALE = 1.0 / float(np.sqrt(HD))

# bf16 consts column layout
C_RT = 0          # [128]  RoPE rotation (R.T)
C_ID = 128        # [128]  identity
C_ONES = 256      # [128]  all-ones
C_COS = 384       # [2048] cos, repeated x2 along d
C_BM = 2432       # [4*512] causal band masks, multiplicative 0/1
NC16 = 4480

_CACHE = {}


def _build():
    nc = bacc.Bacc()
    # All pre-swizzled on the host: per-partition rows are contiguous.
    xh = nc.dram_tensor("xh", [P, NS, NC, 512], BF16, kind="ExternalInput")
    wqh = nc.dram_tensor("wqh", [P, NC, GD], BF16, kind="ExternalInput")
    wkh = nc.dram_tensor("wkh", [P, NC, HD], BF16, kind="ExternalInput")
    wvh = nc.dram_tensor("wvh", [P, NC, HD], BF16, kind="ExternalInput")
    woh = nc.dram_tensor("woh", [P, HPG, DIM], BF16, kind="ExternalInput")
    c16 = nc.dram_tensor("c16", [P, NC16], BF16, kind="ExternalInput")
    c32 = nc.dram_tensor("c32", [P, S], F32, kind="ExternalInput")
    out = nc.dram_tensor("out", [S, DIM], BF16, kind="ExternalOutput")

    EXP = mybir.ActivationFunctionType.Exp

    with tile.TileContext(nc) as tc, ExitStack() as ctx:
        cpool = ctx.enter_context(tc.tile_pool(name="consts", bufs=1))
        persist = ctx.enter_context(tc.tile_pool(name="persist", bufs=1))
        xpool = ctx.enter_context(tc.tile_pool(name="xs", bufs=2))
        vtpool = ctx.enter_context(tc.tile_pool(name="vt", bufs=2))
        tmpp = ctx.enter_context(tc.tile_pool(name="ropetmp", bufs=4))
        ptp = ctx.enter_context(tc.tile_pool(name="pt", bufs=5))
        dsp = ctx.enter_context(tc.tile_pool(name="ds", bufs=4))
        recp = ctx.enter_context(tc.tile_pool(name="rec", bufs=2))
        outp = ctx.enter_context(tc.tile_pool(name="outp", bufs=4))
        # PSUM: psP 2x[P,512] (projection chains + softmax denominator),
        # psQ 2x[P,1024] (QK score pairs; also out-proj), psV 2x[P,512]
        # (PV accumulators; also RoPE rot + V-transpose) = 8 banks exactly.
        psP = ctx.enter_context(tc.tile_pool(name="psP", bufs=2, space="PSUM"))
        psQ = ctx.enter_context(tc.tile_pool(name="psQ", bufs=2, space="PSUM"))
        psV = ctx.enter_context(tc.tile_pool(name="psV", bufs=2, space="PSUM"))

        c16_sb = cpool.tile([P, NC16], BF16, name="c16_sb")
        sin_sb = cpool.tile([P, S], F32, name="sin_sb")
        rt = c16_sb[:, C_RT:C_RT + 128]
        ident = c16_sb[:, C_ID:C_ID + 128]
        ones = c16_sb[:, C_ONES:C_ONES + 128]
        cosf = c16_sb[:, C_COS:C_COS + S]
        bm = c16_sb[:, C_BM:C_BM + 2048]

        wq_sb = persist.tile([P, NC, GD], BF16, name="wq_sb")
        wk_sb = persist.tile([P, NC, HD], BF16, name="wk_sb")
        wv_sb = persist.tile([P, NC, HD], BF16, name="wv_sb")
        wo_sb = persist.tile([P, HPG, DIM], BF16, name="wo_sb")
        q_sb = persist.tile([P, HPG, S], BF16, name="q_sb")
        k_sb = persist.tile([P, S], BF16, name="k_sb")
        v_sb = persist.tile([P, NKT, HD], BF16, name="v_sb")
        attn_sb = persist.tile([P, HPG, S], BF16, name="attn_sb")
        warm_sb = persist.tile([P, 512], BF16, name="warm_sb")

        # ---- warmup: keep PE busy (HAM un-throttle) while first DMAs land
        nc.gpsimd.memset(warm_sb, 0.0)
        for _ in range(6):
            wps = psQ.tile([P, 512], F32, name="warm_ps", tag="qk")
            nc.tensor.matmul(wps, warm_sb[:, :P], warm_sb, start=True, stop=True)

        # ---- startup DMAs; four queues in parallel ----
        # sync: x slice 0 (chunk 0 first for the earliest matmul)
        xs_j = {}
        xs0 = xpool.tile([P, NC, 512], BF16, name="xs", tag="xs")
        nc.sync.dma_start(out=xs0[:, 0:4, :], in_=xh[:, 0, 0:4, :])
        nc.sync.dma_start(out=xs0[:, 4:, :], in_=xh[:, 0, 4:, :])
        xs_j[0] = xs0
        # gpsimd: K/V weights (first projection targets), then Q weights
        nc.gpsimd.dma_start(out=wk_sb, in_=wkh[:, :, :])
        nc.gpsimd.dma_start(out=wv_sb, in_=wvh[:, :, :])
        nc.gpsimd.dma_start(out=wq_sb[:, 0:8, :], in_=wqh[:, 0:8, :])
        nc.gpsimd.dma_start(out=wq_sb[:, 8:, :], in_=wqh[:, 8:, :])
        # scalar: rope/attention consts, then output weights + band masks
        nc.scalar.dma_start(out=c16_sb[:, 0:C_COS + 512],
                            in_=c16[:, 0:C_COS + 512])
        nc.scalar.dma_start(out=sin_sb[:, 0:512], in_=c32[:, 0:512])
        nc.scalar.dma_start(out=wo_sb, in_=woh[:, :, :])
        nc.scalar.dma_start(out=c16_sb[:, C_BM:C_BM + 2048],
                            in_=c16[:, C_BM:C_BM + 2048])

        dma_engs = [nc.sync, nc.gpsimd, nc.scalar]
        out_dma_rr = [0]

        def rope(src):
            t2 = tmpp.tile([P, 512], F32, name="t2")
            nc.vector.tensor_mul(t2, src, cosf[:, sl])
            rot = psV.tile([P, 512], F32, name="rot", tag="pv")
            nc.tensor.matmul(rot, rt, src, start=True, stop=True)
            t1 = tmpp.tile([P, 512], F32, name="t1")
            nc.vector.tensor_mul(t1, rot, sin_sb[:, sl])
            nc.vector.tensor_add(src, t1, t2)

        for j in range(NS):
            sl = slice(512 * j, 512 * (j + 1))
            xs = xs_j[j]

            # prefetch x for slice j+1 (sync queue, behind slice j)
            if j + 1 < NS:
                t = xpool.tile([P, NC, 512], BF16, name="xs", tag="xs")
                nc.sync.dma_start(out=t, in_=xh[:, j + 1, :, :])
                xs_j[j + 1] = t
                nc.scalar.dma_start(
                    out=c16_sb[:, C_COS + 512 * (j + 1):C_COS + 512 * (j + 2)],
                    in_=c16[:, C_COS + 512 * (j + 1):C_COS + 512 * (j + 2)])
                nc.scalar.dma_start(
                    out=sin_sb[:, 512 * (j + 1):512 * (j + 2)],
                    in_=c32[:, 512 * (j + 1):512 * (j + 2)])

            # ---- K projection + RoPE(k) ----
            ps_k = psP.tile([P, 512], F32, name="psp", tag="pp")
            for c in range(NC):
                nc.tensor.matmul(ps_k, wk_sb[:, c, :], xs[:, c, :],
                                 start=(c == 0), stop=(c == NC - 1))
            nc.scalar.copy(k_sb[:, sl], ps_k)
            rope(k_sb[:, sl])

            # ---- V projection + transpose ----
            ps_v = psP.tile([P, 512], F32, name="psp", tag="pp")
            for c in range(NC):
                nc.tensor.matmul(ps_v, wv_sb[:, c, :], xs[:, c, :],
                                 start=(c == 0), stop=(c == NC - 1))
            vt = vtpool.tile([P, 512], BF16, name="vt_sb")
            nc.scalar.copy(vt, ps_v)
            for i in range(4):
                tr = psV.tile([P, 512], BF16, name="tr", tag="pv")
                nc.tensor.transpose(tr[:, :P], vt[:, P * i:P * (i + 1)], ident)
                nc.vector.tensor_copy(v_sb[:, 4 * j + i, :], tr[:, :P])

            # ---- Q projections + RoPE(q), per head ----
            for t in range(HPG):
                ps_q = psP.tile([P, 512], F32, name="psp", tag="pp")
                for c in range(NC):
                    nc.tensor.matmul(ps_q, wq_sb[:, c, 128 * t:128 * (t + 1)],
                                     xs[:, c, :],
                                     start=(c == 0), stop=(c == NC - 1))
                nc.scalar.copy(q_sb[:, t, sl], ps_q)
                rope(q_sb[:, t, sl])

            # ---- attention for slice j ----
            nkt = 4 * (j + 1)
            npair = nkt // 2
            for h in range(HPG):
                q_hi = q_sb[:, h, sl]
                pv = psV.tile([P, 512], F32, name="pv", tag="pv")
                den = psP.tile([P, 512], F32, name="den", tag="pp")
                # pairs of k-tiles; diagonal (masked) pairs first so their
                # longer exp+mask chains hide behind later matmuls
                pairs = ([(4 * j, 4 * j + 1, 0), (4 * j + 2, 4 * j + 3, 1)]
                         + [(2 * m, 2 * m + 1, None) for m in range(2 * j)])
                pts = [None] * npair
                dsums = [None] * npair

                def score(p):
                    kt0, kt1, bi = pairs[p]
                    qk = psQ.tile([P, 1024], F32, name="qk", tag="qk")
                    nc.tensor.matmul(qk[:, 0:512], k_sb[:, P * kt0:P * (kt0 + 1)],
                                     q_hi, start=True, stop=True)
                    nc.tensor.matmul(qk[:, 512:1024], k_sb[:, P * kt1:P * (kt1 + 1)],
                                     q_hi, start=True, stop=True)
                    pt = ptp.tile([P, 1024], BF16, name="pt")
                    nc.scalar.activation(pt, qk, EXP, scale=SCALE)
                    if bi is not None:
                        nc.vector.tensor_mul(pt, pt,
                                             bm[:, 1024 * bi:1024 * (bi + 1)])
                    pts[p] = pt

                def accum(p):
                    kt0, kt1, bi = pairs[p]
                    for z, kt in enumerate((kt0, kt1)):
                        r = kt - 4 * j
                        lo = 128 * r if (bi is not None and r >= 1) else 0
                        nc.tensor.matmul(pv[:, lo:], v_sb[:, kt, :],
                                         pts[p][:, 512 * z + lo:512 * (z + 1)],
                                         start=(p == 0 and z == 0),
                                         stop=(p == npair - 1 and z == 1))
                    ds = dsp.tile([P, 512], BF16, name="ds")
                    nc.vector.tensor_add(ds, pts[p][:, 0:512], pts[p][:, 512:1024])
                    dsums[p] = ds
                    if p % 2 == 1:
                        sq = dsp.tile([P, 512], BF16, name="sq")
                        nc.vector.tensor_add(sq, dsums[p - 1], ds)
                        nc.tensor.matmul(den, ones, sq,
                                         start=(p == 1), stop=(p == npair - 1))

                score(0)
                for p in range(1, npair):
                    score(p)
                    accum(p - 1)
                accum(npair - 1)

                rec = recp.tile([P, 512], F32, name="rec")
                nc.vector.reciprocal_approx_fast(rec, den)
                nc.vector.tensor_mul(attn_sb[:, h, sl], pv, rec)

            # ---- output projection for the 4 s-tiles of this slice ----
            for st in range(4 * j, 4 * (j + 1)):
                for e in range(NS):
                    ops = psQ.tile([P, 512], F32, name="ops", tag="qk")
                    for hc in range(HPG):
                        nc.tensor.matmul(
                            ops, attn_sb[:, hc, P * st:P * (st + 1)],
                            wo_sb[:, hc, 512 * e:512 * (e + 1)],
                            start=(hc == 0), stop=(hc == HPG - 1))
                    osb = outp.tile([P, 512], BF16, name="osb")
                    nc.vector.tensor_copy(osb, ops)
                    eng = dma_engs[out_dma_rr[0] % 3]
                    out_dma_rr[0] += 1
                    eng.dma_start(
                        out=out[P * st:P * (st + 1), 512 * e:512 * (e + 1)],
                        in_=osb)

    nc.compile()
    return nc


def _consts16(freqs_cos):
    c = np.zeros((P, NC16), np.float32)
    rtm = np.zeros((P, P), np.float32)
    idx = np.arange(0, P, 2)
    rtm[idx, idx + 1] = 1.0    # (R.T)[2j, 2j+1] = +1
    rtm[idx + 1, idx] = -1.0   # (R.T)[2j+1, 2j] = -1
    c[:, C_RT:C_RT + P] = rtm
    c[:, C_ID:C_ID + P] = np.eye(P, dtype=np.float32)
    c[:, C_ONES:C_ONES + P] = 1.0
    c[:, C_COS:C_COS + S] = np.repeat(np.asarray(freqs_cos, np.float32).T, 2,
                                      axis=0)
    ki = np.arange(P)[:, None]
    qi = np.arange(512)[None, :]
    for r in range(4):
        c[:, C_BM + 512 * r:C_BM + 512 * (r + 1)] = \
            (ki <= qi - P * r).astype(np.float32)
    return c.astype(BF)


def _swiz_w(wT, width):
    # [DIM, width] -> [P, NC, width] with [p, c, :] = wT[128c + p, :]
    return np.ascontiguousarray(
        wT.reshape(NC, P, width).transpose(1, 0, 2)).astype(BF)


def _in_maps(x, wq, wk, wv, wo, freqs_cos, freqs_sin):
    x = np.asarray(x, np.float32)
    wq = np.asarray(wq, np.float32)
    wk = np.asarray(wk, np.float32)
    wv = np.asarray(wv, np.float32)
    wo = np.asarray(wo, np.float32)
    c16a = _consts16(freqs_cos)
    c32a = np.ascontiguousarray(
        np.repeat(np.asarray(freqs_sin, np.float32).T, 2, axis=0))
    xhs = []
    for b in range(B):
        xT = x[b].T  # [DIM, S]
        # [p, j, c, s] = xT[128c + p, 512j + s]
        xhs.append(np.ascontiguousarray(
            xT.reshape(NC, P, NS, 512).transpose(1, 2, 0, 3)).astype(BF))
    maps = []
    for core in range(8):
        b, g = divmod(core, 4)
        maps.append({
            "xh": xhs[b],
            "wqh": _swiz_w(np.ascontiguousarray(wq[GD * g:GD * (g + 1), :].T), GD),
            "wkh": _swiz_w(np.ascontiguousarray(wk[HD * g:HD * (g + 1), :].T), HD),
            "wvh": _swiz_w(np.ascontiguousarray(wv[HD * g:HD * (g + 1), :].T), HD),
            "woh": np.ascontiguousarray(
                wo[:, GD * g:GD * (g + 1)].T.reshape(HPG, P, DIM)
                .transpose(1, 0, 2)).astype(BF),
            "c16": c16a,
            "c32": c32a,
        })
    return maps


def _get_nc():
    if "nc" not in _CACHE:
        _CACHE["nc"] = _build()
    return _CACHE["nc"]


def _run(in_maps, trace=False):
    return run_bass_kernel_spmd(_get_nc(), in_maps, core_ids=list(range(8)),
                                trace=trace)


def kernel(x, wq, wk, wv, wo, freqs_cos, freqs_sin):
    res = _run(_in_maps(x, wq, wk, wv, wo, freqs_cos, freqs_sin))
    out = np.zeros((B, S, DIM), np.float32)
    for core in range(8):
        b = core // 4
        out[b] += res.results[core]["out"].astype(np.float32)
    return out


# revision 11
# speedup vs baseline: 1.1774x; 1.0106x over previous
"""GQA causal attention (B=2, S=2048, D=2048, 16 q heads / 4 kv heads, RoPE)
for 8 Trainium2 NeuronCores.

Sharding: core i = (batch b = i//4, kv-head group g = i%4). Each core computes
its group's Q/K/V projections, RoPE, causal attention and the partial output
projection; the host sums the 4 per-group partials per batch.

On-core layout is fully "transposed" (features on partitions):
  xT [D, S], QT/KT [d, S] -> QK scores land as [k, q], softmax runs along k
  (partitions) with the denominator computed by an all-ones matmul, and PV
  accumulates out^T [d, q] directly in PSUM. The final projection contracts
  over the group's 512 head-dims on partitions.

Everything is bf16 except PSUM accumulation, the exp input (fp32 scores in
PSUM) and the RoPE temporaries. The whole kernel is one fused pipeline:
per 512-token slice j we emit {K proj, RoPE(k), V proj, V transpose,
Q proj + RoPE(q) per head, attention over k-tiles 0..4(j+1), output
projection}, so the tensor engine never drains between "phases". QK scores
for two k-tiles share one 2-bank PSUM tile so exp runs 1024 wide; softmax
denominators are tree-summed on the DVE (4 tiles -> 1 ones-matmul). All
HBM operands are pre-swizzled on the host so every DMA moves 4-16KB of
contiguous bytes per partition row.
"""

import sys
import types

sys.path.insert(0, "/opt/trn_rl_repo")

# If tracing is ever requested (e.g. BASS_TRACE=1 in the environment),
# concourse needs antenv.axon_hooks, which this image lacks; provide it.
try:
    import antenv  # noqa: F401

    if "antenv.axon_hooks" not in sys.modules:
        from trn_agent_boot.trn_boot import _ntff_profile_via_ctypes

        _mod = types.ModuleType("antenv.axon_hooks")
        _hook = _ntff_profile_via_ctypes("/opt/axon/libaxon_pjrt.so")
        _mod.get_axon_ntff_profile_hook = lambda: _hook
        sys.modules["antenv.axon_hooks"] = _mod
except Exception:
    pass

import numpy as np
import ml_dtypes
from contextlib import ExitStack

import concourse.bacc as bacc
import concourse.mybir as mybir
import concourse.tile as tile
from concourse.bass_utils import run_bass_kernel_spmd

B, S, DIM = 2, 2048, 2048
N_HEADS, N_KV, HD = 16, 4, 128
HPG = N_HEADS // N_KV      # q heads per kv group
GD = HPG * HD              # 512 = group width
P = 128
NS = S // 512              # 4 s-slices of 512
NC = DIM // P              # 16 contraction chunks of 128
NKT = S // P               # 16 k tiles
F32 = mybir.dt.float32
BF16 = mybir.dt.bfloat16
BF = ml_dtypes.bfloat16
SCALE = 1.0 / float(np.sqrt(HD))

# bf16 consts column layout
C_RT = 0          # [128]  RoPE rotation (R.T)
C_ID = 128        # [128]  identity
C_ONES = 256      # [128]  all-ones
C_COS = 384       # [2048] cos, repeated x2 along d
C_BM = 2432       # [4*512] causal band masks, multiplicative 0/1
NC16 = 4480

_CACHE = {}


def _build():
    nc = bacc.Bacc()
    # All pre-swizzled on the host: per-partition rows are contiguous.
    xh = nc.dram_tensor("xh", [P, NS, NC, 512], BF16, kind="ExternalInput")
    wqh = nc.dram_tensor("wqh", [P, HPG, NC, HD], BF16, kind="ExternalInput")
    wkh = nc.dram_tensor("wkh", [P, NC, HD], BF16, kind="ExternalInput")
    wvh = nc.dram_tensor("wvh", [P, NC, HD], BF16, kind="ExternalInput")
    woh = nc.dram_tensor("woh", [P, HPG, DIM], BF16, kind="ExternalInput")
    c16 = nc.dram_tensor("c16", [P, NC16], BF16, kind="ExternalInput")
    c32 = nc.dram_tensor("c32", [P, S], F32, kind="ExternalInput")
    out = nc.dram_tensor("out", [S, DIM], BF16, kind="ExternalOutput")

    EXP = mybir.ActivationFunctionType.Exp

    with tile.TileContext(nc) as tc, ExitStack() as ctx:
        cpool = ctx.enter_context(tc.tile_pool(name="consts", bufs=1))
        persist = ctx.enter_context(tc.tile_pool(name="persist", bufs=1))
        xpool = ctx.enter_context(tc.tile_pool(name="xs", bufs=2))
        vtpool = ctx.enter_context(tc.tile_pool(name="vt", bufs=2))
        tmpp = ctx.enter_context(tc.tile_pool(name="ropetmp", bufs=4))
        ptp = ctx.enter_context(tc.tile_pool(name="pt", bufs=5))
        dsp = ctx.enter_context(tc.tile_pool(name="ds", bufs=4))
        recp = ctx.enter_context(tc.tile_pool(name="rec", bufs=2))
        outp = ctx.enter_context(tc.tile_pool(name="outp", bufs=4))
        # PSUM: psP 2x[P,512] (projection chains + softmax denominator),
        # psQ 2x[P,1024] (QK score pairs; also out-proj), psV 2x[P,512]
        # (PV accumulators; also RoPE rot + V-transpose) = 8 banks exactly.
        psP = ctx.enter_context(tc.tile_pool(name="psP", bufs=2, space="PSUM"))
        psQ = ctx.enter_context(tc.tile_pool(name="psQ", bufs=2, space="PSUM"))
        psV = ctx.enter_context(tc.tile_pool(name="psV", bufs=2, space="PSUM"))

        c16_sb = cpool.tile([P, NC16], BF16, name="c16_sb")
        sin_sb = cpool.tile([P, S], F32, name="sin_sb")
        rt = c16_sb[:, C_RT:C_RT + 128]
        ident = c16_sb[:, C_ID:C_ID + 128]
        ones = c16_sb[:, C_ONES:C_ONES + 128]
        cosf = c16_sb[:, C_COS:C_COS + S]
        bm = c16_sb[:, C_BM:C_BM + 2048]

        wq_sb = persist.tile([P, HPG, NC, HD], BF16, name="wq_sb")
        wk_sb = persist.tile([P, NC, HD], BF16, name="wk_sb")
        wv_sb = persist.tile([P, NC, HD], BF16, name="wv_sb")
        wo_sb = persist.tile([P, HPG, DIM], BF16, name="wo_sb")
        q_sb = persist.tile([P, HPG, S], BF16, name="q_sb")
        k_sb = persist.tile([P, S], BF16, name="k_sb")
        v_sb = persist.tile([P, NKT, HD], BF16, name="v_sb")
        attn_sb = persist.tile([P, HPG, S], BF16, name="attn_sb")
        warm_sb = persist.tile([P, 512], BF16, name="warm_sb")

        # ---- warmup: keep PE busy (HAM un-throttle) while first DMAs land
        nc.vector.memset(warm_sb, 0.0)
        for _ in range(24):
            wps = psQ.tile([P, 512], F32, name="warm_ps", tag="qk")
            nc.tensor.matmul(wps, warm_sb[:, :P], warm_sb, start=True, stop=True)

        # ---- startup DMAs; x chunks + weights spread over the three DMA
        # queues in projection-consumption order (K, Q0.., V), per-chunk so
        # the K chain unblocks incrementally
        xs_j = {}
        xs0 = xpool.tile([P, NC, 512], BF16, name="xs", tag="xs")
        xs_j[0] = xs0
        for cc in range(0, 6):
            nc.sync.dma_start(out=xs0[:, cc, :], in_=xh[:, 0, cc, :])
        nc.gpsimd.dma_start(out=wk_sb, in_=wkh[:, :, :])
        for cc in range(6, 11):
            nc.gpsimd.dma_start(out=xs0[:, cc, :], in_=xh[:, 0, cc, :])
        nc.scalar.dma_start(out=c16_sb[:, 0:C_COS + 512],
                            in_=c16[:, 0:C_COS + 512])
        nc.scalar.dma_start(out=sin_sb[:, 0:512], in_=c32[:, 0:512])
        nc.scalar.dma_start(out=wq_sb[:, 0], in_=wqh[:, 0])
        for cc in range(11, 16):
            nc.scalar.dma_start(out=xs0[:, cc, :], in_=xh[:, 0, cc, :])
        # remaining weights, roughly in consumption order
        nc.sync.dma_start(out=wq_sb[:, 1], in_=wqh[:, 1])
        nc.gpsimd.dma_start(out=wq_sb[:, 2], in_=wqh[:, 2])
        nc.gpsimd.dma_start(out=wq_sb[:, 3], in_=wqh[:, 3])
        nc.gpsimd.dma_start(out=wv_sb, in_=wvh[:, :, :])
        nc.scalar.dma_start(out=wo_sb, in_=woh[:, :, :])
        nc.scalar.dma_start(out=c16_sb[:, C_BM:C_BM + 2048],
                            in_=c16[:, C_BM:C_BM + 2048])

        dma_engs = [nc.sync, nc.gpsimd, nc.scalar]
        out_dma_rr = [0]

        def rope(src):
            t2 = tmpp.tile([P, 512], F32, name="t2")
            nc.vector.tensor_mul(t2, src, cosf[:, sl])
            rot = psV.tile([P, 512], F32, name="rot", tag="pv")
            nc.tensor.matmul(rot, rt, src, start=True, stop=True)
            t1 = tmpp.tile([P, 512], F32, name="t1")
            nc.vector.tensor_mul(t1, rot, sin_sb[:, sl])
            nc.vector.tensor_add(src, t1, t2)

        for j in range(NS):
            sl = slice(512 * j, 512 * (j + 1))
            xs = xs_j[j]

            # prefetch x for slice j+1 (sync queue, behind slice j)
            if j + 1 < NS:
                t = xpool.tile([P, NC, 512], BF16, name="xs", tag="xs")
                nc.sync.dma_start(out=t, in_=xh[:, j + 1, :, :])
                xs_j[j + 1] = t
                nc.scalar.dma_start(
                    out=c16_sb[:, C_COS + 512 * (j + 1):C_COS + 512 * (j + 2)],
                    in_=c16[:, C_COS + 512 * (j + 1):C_COS + 512 * (j + 2)])
                nc.scalar.dma_start(
                    out=sin_sb[:, 512 * (j + 1):512 * (j + 2)],
                    in_=c32[:, 512 * (j + 1):512 * (j + 2)])

            # ---- K projection + RoPE(k) ----
            ps_k = psP.tile([P, 512], F32, name="psp", tag="pp")
            for c in range(NC):
                nc.tensor.matmul(ps_k, wk_sb[:, c, :], xs[:, c, :],
                                 start=(c == 0), stop=(c == NC - 1))
            nc.scalar.copy(k_sb[:, sl], ps_k)
            rope(k_sb[:, sl])

            # ---- Q projections + RoPE(q), per head ----
            for t in range(HPG):
                ps_q = psP.tile([P, 512], F32, name="psp", tag="pp")
                for c in range(NC):
                    nc.tensor.matmul(ps_q, wq_sb[:, t, c, :], xs[:, c, :],
                                     start=(c == 0), stop=(c == NC - 1))
                nc.scalar.copy(q_sb[:, t, sl], ps_q)
                rope(q_sb[:, t, sl])

            # ---- V projection + transpose (V weights arrive last) ----
            ps_v = psP.tile([P, 512], F32, name="psp", tag="pp")
            for c in range(NC):
                nc.tensor.matmul(ps_v, wv_sb[:, c, :], xs[:, c, :],
                                 start=(c == 0), stop=(c == NC - 1))
            vt = vtpool.tile([P, 512], BF16, name="vt_sb")
            nc.scalar.copy(vt, ps_v)
            for i in range(4):
                tr = psV.tile([P, 512], BF16, name="tr", tag="pv")
                nc.tensor.transpose(tr[:, :P], vt[:, P * i:P * (i + 1)], ident)
                nc.vector.tensor_copy(v_sb[:, 4 * j + i, :], tr[:, :P])

            # ---- attention for slice j ----
            nkt = 4 * (j + 1)
            npair = nkt // 2
            for h in range(HPG):
                q_hi = q_sb[:, h, sl]
                pv = psV.tile([P, 512], F32, name="pv", tag="pv")
                den = psP.tile([P, 512], F32, name="den", tag="pp")
                # pairs of k-tiles; diagonal (masked) pairs first so their
                # longer exp+mask chains hide behind later matmuls
                pairs = ([(4 * j, 4 * j + 1, 0), (4 * j + 2, 4 * j + 3, 1)]
                         + [(2 * m, 2 * m + 1, None) for m in range(2 * j)])
                pts = [None] * npair
                dsums = [None] * npair

                def score(p):
                    kt0, kt1, bi = pairs[p]
                    qk = psQ.tile([P, 1024], F32, name="qk", tag="qk")
                    nc.tensor.matmul(qk[:, 0:512], k_sb[:, P * kt0:P * (kt0 + 1)],
                                     q_hi, start=True, stop=True)
                    nc.tensor.matmul(qk[:, 512:1024], k_sb[:, P * kt1:P * (kt1 + 1)],
                                     q_hi, start=True, stop=True)
                    pt = ptp.tile([P, 1024], BF16, name="pt")
                    nc.scalar.activation(pt, qk, EXP, scale=SCALE)
                    if bi is not None:
                        nc.vector.tensor_mul(pt, pt,
                                             bm[:, 1024 * bi:1024 * (bi + 1)])
                    pts[p] = pt

                def accum(p):
                    kt0, kt1, bi = pairs[p]
                    for z, kt in enumerate((kt0, kt1)):
                        r = kt - 4 * j
                        lo = 128 * r if (bi is not None and r >= 1) else 0
                        nc.tensor.matmul(pv[:, lo:], v_sb[:, kt, :],
                                         pts[p][:, 512 * z + lo:512 * (z + 1)],
                                         start=(p == 0 and z == 0),
                                         stop=(p == npair - 1 and z == 1))
                    ds = dsp.tile([P, 512], BF16, name="ds")
                    nc.vector.tensor_add(ds, pts[p][:, 0:512], pts[p][:, 512:1024])
                    dsums[p] = ds
                    if p % 2 == 1:
                        sq = dsp.tile([P, 512], BF16, name="sq")
                        nc.vector.tensor_add(sq, dsums[p - 1], ds)
                        nc.tensor.matmul(den, ones, sq,
                                         start=(p == 1), stop=(p == npair - 1))

                score(0)
                for p in range(1, npair):
                    score(p)
                    accum(p - 1)
                accum(npair - 1)

                rec = recp.tile([P, 512], F32, name="rec")
                nc.vector.reciprocal_approx_fast(rec, den)
                nc.vector.tensor_mul(attn_sb[:, h, sl], pv, rec)

            # ---- output projection for the 4 s-tiles of this slice ----
            for st in range(4 * j, 4 * (j + 1)):
                for e in range(NS):
                    ops = psQ.tile([P, 512], F32, name="ops", tag="qk")
                    for hc in range(HPG):
                        nc.tensor.matmul(
                            ops, attn_sb[:, hc, P * st:P * (st + 1)],
                            wo_sb[:, hc, 512 * e:512 * (e + 1)],
                            start=(hc == 0), stop=(hc == HPG - 1))
                    osb = outp.tile([P, 512], BF16, name="osb")
                    nc.vector.tensor_copy(osb, ops)
                    eng = dma_engs[out_dma_rr[0] % 3]
                    out_dma_rr[0] += 1
                    eng.dma_start(
                        out=out[P * st:P * (st + 1), 512 * e:512 * (e + 1)],
                        in_=osb)

    nc.compile()
    return nc


def _consts16(freqs_cos):
    c = np.zeros((P, NC16), np.float32)
    rtm = np.zeros((P, P), np.float32)
    idx = np.arange(0, P, 2)
    rtm[idx, idx + 1] = 1.0    # (R.T)[2j, 2j+1] = +1
    rtm[idx + 1, idx] = -1.0   # (R.T)[2j+1, 2j] = -1
    c[:, C_RT:C_RT + P] = rtm
    c[:, C_ID:C_ID + P] = np.eye(P, dtype=np.float32)
    c[:, C_ONES:C_ONES + P] = 1.0
    c[:, C_COS:C_COS + S] = np.repeat(np.asarray(freqs_cos, np.float32).T, 2,
                                      axis=0)
    ki = np.arange(P)[:, None]
    qi = np.arange(512)[None, :]
    for r in range(4):
        c[:, C_BM + 512 * r:C_BM + 512 * (r + 1)] = \
            (ki <= qi - P * r).astype(np.float32)
    return c.astype(BF)


def _swiz_w(wT, width):
    # [DIM, width] -> [P, NC, width] with [p, c, :] = wT[128c + p, :]
    return np.ascontiguousarray(
        wT.reshape(NC, P, width).transpose(1, 0, 2)).astype(BF)


def _in_maps(x, wq, wk, wv, wo, freqs_cos, freqs_sin):
    x = np.asarray(x, np.float32)
    wq = np.asarray(wq, np.float32)
    wk = np.asarray(wk, np.float32)
    wv = np.asarray(wv, np.float32)
    wo = np.asarray(wo, np.float32)
    c16a = _consts16(freqs_cos)
    c32a = np.ascontiguousarray(
        np.repeat(np.asarray(freqs_sin, np.float32).T, 2, axis=0))
    xhs = []
    for b in range(B):
        xT = x[b].T  # [DIM, S]
        # [p, j, c, s] = xT[128c + p, 512j + s]
        xhs.append(np.ascontiguousarray(
            xT.reshape(NC, P, NS, 512).transpose(1, 2, 0, 3)).astype(BF))
    maps = []
    for core in range(8):
        b, g = divmod(core, 4)
        wqT = np.ascontiguousarray(wq[GD * g:GD * (g + 1), :].T)  # [DIM, GD]
        maps.append({
            "xh": xhs[b],
            "wqh": np.ascontiguousarray(
                wqT.reshape(NC, P, HPG, HD).transpose(1, 2, 0, 3)).astype(BF),
            "wkh": _swiz_w(np.ascontiguousarray(wk[HD * g:HD * (g + 1), :].T), HD),
            "wvh": _swiz_w(np.ascontiguousarray(wv[HD * g:HD * (g + 1), :].T), HD),
            "woh": np.ascontiguousarray(
                wo[:, GD * g:GD * (g + 1)].T.reshape(HPG, P, DIM)
                .transpose(1, 0, 2)).astype(BF),
            "c16": c16a,
            "c32": c32a,
        })
    return maps


def _get_nc():
    if "nc" not in _CACHE:
        _CACHE["nc"] = _build()
    return _CACHE["nc"]


def _run(in_maps, trace=False):
    return run_bass_kernel_spmd(_get_nc(), in_maps, core_ids=list(range(8)),
                                trace=trace)


def kernel(x, wq, wk, wv, wo, freqs_cos, freqs_sin):
    res = _run(_in_maps(x, wq, wk, wv, wo, freqs_cos, freqs_sin))
    out = np.zeros((B, S, DIM), np.float32)
    for core in range(8):
        b = core // 4
        out[b] += res.results[core]["out"].astype(np.float32)
    return out
